# revision 22
# baseline (speedup 1.0000x reference)
"""PiLoraLayer TRN2 kernel: y = x + (alpha/r) * sin((2/pi) * (x @ A) @ B).

x: [4, 4096, 4096] f32; A = A_int8 * scale_A (per-col), B = B_int8 * scale_B
(per-col); rank 16 bottleneck.

Strategy (data-parallel over 8 NeuronCores, fp16 in / fp16-sine out):
- Host: cast x to fp16 and PRE-TRANSPOSE each core's [2048, 4096] token shard
  to hidden-major layout [quarter, partition, k-chunk, token] so the hidden
  dim lands on SBUF partitions for mm1 (no PE transposes, no PSUM->SBUF
  copies of x).
- Host folds scales into Bp = scale_A[:,None] * B_q * scale_B[None,:] / pi^2
  (f32); A stays as exact int values in fp16. Then u = (x@A)@Bp equals
  arg/(2*pi) and y = x + 2*sin(2*pi*u).
- Device per 512-token quarter:
    - one 4 MB fp16 DMA in (partition-contiguous, prearranged on host)
    - mm1 (fp16): h1_ps[16, 512] += A_k^T @ xT_k over 32 hidden chunks;
      DVE copies h1 to SBUF f32r.
    - mm2 (f32r, natural out): per (token-chunk c, hidden-block ub):
      u[128, 1024] = h1_c^T @ Bp_block in two N=512 matmuls.
    - range reduction in ONE custom DVE op (FRAC_RNE_ANT, registered below):
      frac = u - ((u + 1.5*2^23) - 1.5*2^23)  [f32 RNE rounds to nearest
      integer] -> fp16 SBUF. This removes the negative-identity -k matmul
      accumulate from the PE entirely (no extra matmuls, no LDWEIGHTS churn,
      and the PSUM bank is freed right after the DVE pass).
    - ACT sin: s = sin(2*pi*frac) from fp16 SBUF -> fp16 SBUF (natural
      [token, hidden] layout)
    - one 4 MB fp16 DMA out of s per quarter
- Host: y = x_f32 + 2 * s (residual add in f32 on host: removes a device DVE
  pass, halves output DMA vs f32 y, and keeps the residual path exact).
"""

import sys

sys.path.insert(0, "/opt/trn_rl_repo")

import numpy as np

import concourse.bacc as bacc
import concourse.bass as bass
import concourse.dve_ops as dve_ops
import concourse.tile as tile
from concourse import mybir
from concourse.bass import ts
from concourse.bass_utils import run_bass_kernel_spmd
from concourse.dve_ops import DveOp
from concourse.dve_spec import Spec, Src0, C0, C1
from concourse.dve_table_gen import dve_ver_for
from concourse.dve_uop import DveOpSpec

P = 128
HIDDEN = 4096
RANK = 16
N_CORES = 8
TOTAL_ROWS = 4 * 4096
ROWS = TOTAL_ROWS // N_CORES  # 2048 tokens per core
T = 512  # steady-state tokens per quarter (pipeline unit)
TEDGE = 256  # first/last quarter size: halves pipeline fill + drain
KC = HIDDEN // P  # 32 hidden chunks
UBLK = 1024  # hidden block width of one u tile (2 PSUM banks)
NUB = HIDDEN // UBLK  # 4 hidden blocks
MAGIC = 12582912.0  # 1.5 * 2^23: f32 add/sub rounds to nearest integer
SCALE_2PI = 6.283185  # slightly < 2*pi so the Sin LUT arg stays in [-pi, pi]

F32 = mybir.dt.float32
F32R = mybir.dt.float32r
BF16 = mybir.dt.bfloat16
FP16 = mybir.dt.float16


def _frac_ref(in0, in1, s0, s1, imm2):
    a = (in0.astype(np.float32) + np.float32(s0)).astype(np.float32)
    k = (a - np.float32(s1)).astype(np.float32)
    return (in0.astype(np.float32) - k).astype(np.float32)


def _register_frac_op():
    """Register the FRAC_RNE_ANT custom DVE op (one-instruction magic-number
    range reduction: out = in0 - ((in0 + s0) - s1), s0 = s1 = 1.5*2^23)."""
    for op in dve_ops.OPS:
        if op.name == "FRAC_RNE_ANT":
            return op
    spec = Spec(body=Src0 - ((Src0 + C0) - C1), reference=_frac_ref)
    op = DveOp("FRAC_RNE_ANT", spec, subdim=False, uops_sha={})
    dve_ops.OPS.append(op)
    dve_ops.CUSTOM_DVE_SPECS[op.name] = spec
    dve_ops._SUB_OPCODE_FOR_NAME[op.name] = (
        max(dve_ops._SUB_OPCODE_FOR_NAME.values()) + 1
    )
    for trn in ("TRN2",):
        ver = dve_ver_for(trn)
        from concourse.dve_spec import lower

        s = DveOpSpec(
            name=op.name,
            opcode=dve_ops.get_dve_sub_opcode(op.name),
            uops=lower(spec, ver=ver),
            rd1_en=False,
        )
        op.uops_sha[ver] = s.sha(ver)
    return op


FRAC_OP = _register_frac_op()


def _quarter_sizes(rows):
    if rows > 2 * TEDGE and (rows - 2 * TEDGE) % T == 0:
        return [TEDGE] + [T] * ((rows - 2 * TEDGE) // T) + [TEDGE]
    return [T] * (rows // T)


def build_nc(rows: int = ROWS):
    """Per-core Bass program for a [rows, 4096] token shard."""
    sizes = _quarter_sizes(rows)

    nc = bacc.Bacc(
        "TRN2",
        target_bir_lowering=False,
        debug=False,
        enable_asserts=False,
        num_devices=N_CORES,
    )
    # x prearranged on host: [128, KC*rows] fp16; per-quarter blocks of
    # [128, KC*tok] (partition-contiguous), element (p, off_q + k*tok + t) =
    # x[tok0_q + t, k*128 + p] of this core's natural [rows, 4096] shard.
    x_d = nc.dram_tensor("x", [P, KC * rows], FP16, kind="ExternalInput").ap()
    # A prearranged: [128, KC, 16] fp16 (exact int8 values).
    a_d = nc.dram_tensor("A", [P, KC, RANK], FP16, kind="ExternalInput").ap()
    bp_d = nc.dram_tensor("Bp", [RANK, HIDDEN], F32, kind="ExternalInput").ap()
    # s output in NATURAL layout [rows, 4096] fp16.
    s_d = nc.dram_tensor("out", [rows, HIDDEN], FP16, kind="ExternalOutput").ap()

    with tile.TileContext(nc) as tc:
        with (
            tc.tile_pool(name="singles", bufs=1) as singles,
            tc.tile_pool(name="xp", bufs=3) as xpool,
            tc.tile_pool(name="sp", bufs=2) as spool,
            tc.tile_pool(name="fp", bufs=3) as fpool,
            tc.tile_pool(name="h1sb", bufs=2) as h1pool,
            tc.tile_pool(name="h1p", bufs=2, space="PSUM") as h1_psum,
            tc.tile_pool(name="up", bufs=3, space="PSUM") as u_psum,
        ):
            a_sb = singles.tile([P, KC, RANK], FP16)
            nc.sync.dma_start(out=a_sb[:], in_=a_d[:, :, :])
            bp_sb = singles.tile([RANK, HIDDEN], F32R)
            nc.sync.dma_start(out=bp_sb[:], in_=bp_d[:, :].bitcast(F32R))

            def tail_jobs(state):
                """Generator of tail-job closures for a finished quarter."""
                h1_sb, s_sb, _row0, nch = state

                def job(c, ub):
                    u_ps = u_psum.tile([P, UBLK], F32)
                    for jj in range(2):
                        nc.tensor.matmul(
                            u_ps[:, ts(jj, 512)],
                            h1_sb[:, ts(c, P)],
                            bp_sb[:, ub * UBLK + jj * 512 : ub * UBLK + (jj + 1) * 512],
                            start=True,
                            stop=True,
                        )
                    frac = fpool.tile([P, UBLK], FP16)
                    nc.vector._custom_dve(
                        FRAC_OP, out=frac[:], in0=u_ps[:], s0=MAGIC, s1=MAGIC
                    )
                    nc.scalar.activation(
                        out=s_sb[:, c, ts(ub, UBLK)],
                        in_=frac[:],
                        func=mybir.ActivationFunctionType.Sin,
                        scale=SCALE_2PI,
                    )

                for c in range(nch):
                    for ub in range(NUB):
                        yield lambda c=c, ub=ub: job(c, ub)

            def flush_c(prev, c):
                row0, s_sb = prev[2], prev[1]
                r0 = row0 + c * P
                nc.scalar.dma_start(
                    out=s_d[r0 : r0 + P, :].rearrange("(c p) h -> p c h", p=P),
                    in_=s_sb[:, c : c + 1, :],
                )

            prev = None  # (h1_sb, s_sb, row0, nch) of the previous quarter
            prev_jobs = None
            row0 = 0
            for tok in sizes:
                nch = tok // P
                x_sb = xpool.tile([P, KC * tok], FP16)
                off = KC * row0
                npc = 4
                piece = KC * tok // npc
                for pi in range(npc):
                    nc.sync.dma_start(
                        out=x_sb[:, pi * piece : (pi + 1) * piece],
                        in_=x_d[:, off + pi * piece : off + (pi + 1) * piece],
                    )
                s_sb = spool.tile([P, nch, HIDDEN], FP16)
                h1_ps = h1_psum.tile([RANK, tok], F32)
                prev_njobs = prev[3] * NUB if prev is not None else 0
                emitted = 0
                flushed_c = 0
                for k in range(KC):
                    nc.tensor.matmul(
                        h1_ps[:],
                        a_sb[:, k, :],
                        x_sb[:, k * tok : (k + 1) * tok],
                        start=(k == 0),
                        stop=(k == KC - 1),
                    )
                    if prev_jobs is not None:
                        target = (k + 1) * prev_njobs // KC
                        while emitted < target:
                            next(prev_jobs)()
                            emitted += 1
                        while (flushed_c + 1) * NUB <= emitted:
                            flush_c(prev, flushed_c)
                            flushed_c += 1
                h1_sb = h1pool.tile([RANK, tok], F32R)
                nc.vector.tensor_copy(h1_sb[:], h1_ps[:])
                if prev_jobs is not None:
                    while flushed_c < prev[3]:
                        flush_c(prev, flushed_c)
                        flushed_c += 1
                prev = (h1_sb, s_sb, row0, nch)
                prev_jobs = tail_jobs(prev)
                row0 += tok

            # drain: the last quarter's tail has no successor to hide in
            for j, job in enumerate(prev_jobs):
                job()
                if (j + 1) % NUB == 0:
                    flush_c(prev, j // NUB)

    nc.compile()
    return nc


_NC_CACHE: dict[int, object] = {}


def _get_nc(rows: int = ROWS):
    nc = _NC_CACHE.get(rows)
    if nc is None:
        nc = build_nc(rows)
        _NC_CACHE[rows] = nc
    return nc


def _prep_weights(A_int8, B_int8, scale_A, scale_B):
    # A as exact integer values in fp16, prearranged [128, KC, 16]
    a_f = np.ascontiguousarray(
        A_int8.astype(np.float16).reshape(KC, P, RANK).transpose(1, 0, 2)
    )
    bp = np.ascontiguousarray(
        scale_A.astype(np.float32)[:, None]
        * B_int8.astype(np.float32)
        * scale_B.astype(np.float32)[None, :]
        * np.float32(1.0 / (np.pi * np.pi))
    )
    return a_f, bp


def _prearrange_x(x16_shard):
    """[rows, 4096] fp16 -> [128, KC*rows] flat per-quarter blocks."""
    rows = x16_shard.shape[0]
    blocks = []
    r0 = 0
    for tok in _quarter_sizes(rows):
        blk = x16_shard[r0 : r0 + tok].reshape(tok, KC, P).transpose(2, 1, 0)
        blocks.append(np.ascontiguousarray(blk).reshape(P, KC * tok))
        r0 += tok
    return np.ascontiguousarray(np.concatenate(blocks, axis=1))


def kernel(x, A_int8, B_int8, scale_A, scale_B):
    x = np.asarray(x)
    orig_shape = x.shape
    xf = x.reshape(TOTAL_ROWS, HIDDEN)
    x16 = xf.astype(np.float16)
    a_f, bp = _prep_weights(
        np.asarray(A_int8), np.asarray(B_int8), np.asarray(scale_A), np.asarray(scale_B)
    )

    nc = _get_nc(ROWS)
    in_maps = [
        {
            "x": _prearrange_x(x16[i * ROWS : (i + 1) * ROWS]),
            "A": a_f,
            "Bp": bp,
        }
        for i in range(N_CORES)
    ]
    res = run_bass_kernel_spmd(nc, in_maps, core_ids=list(range(N_CORES)))
    y = np.empty((TOTAL_ROWS, HIDDEN), dtype=np.float32)
    for i, r in enumerate(res.results):
        y[i * ROWS : (i + 1) * ROWS] = xf[i * ROWS : (i + 1) * ROWS] + 2.0 * r[
            "out"
        ].astype(np.float32)
    return y.reshape(orig_shape)


# revision 23
# speedup vs baseline: 1.0817x; 1.0817x over previous
"""PiLoraLayer TRN2 kernel: y = x + (alpha/r) * sin((2/pi) * (x @ A) @ B).

x: [4, 4096, 4096] f32; A = A_int8 * scale_A (per-col), B = B_int8 * scale_B
(per-col); rank 16 bottleneck.

Strategy (data-parallel over 8 NeuronCores, fp16 in / fp16-sine out):
- Host: cast x to fp16 and PRE-TRANSPOSE each core's [2048, 4096] token shard
  to hidden-major layout [quarter, partition, k-chunk, token] so the hidden
  dim lands on SBUF partitions for mm1 (no PE transposes, no PSUM->SBUF
  copies of x).
- Host folds scales into Bp = scale_A[:,None] * B_q * scale_B[None,:] / pi^2
  (f32); A stays as exact int values in fp16. Then u = (x@A)@Bp equals
  arg/(2*pi) and y = x + 2*sin(2*pi*u).
- Device per 512-token quarter:
    - one 4 MB fp16 DMA in (partition-contiguous, prearranged on host)
    - mm1 (fp16): h1_ps[16, 512] += A_k^T @ xT_k over 32 hidden chunks;
      DVE copies h1 to SBUF f32r.
    - mm2 (f32r, natural out): per (token-chunk c, hidden-block ub):
      u[128, 1024] = h1_c^T @ Bp_block in two N=512 matmuls.
    - range reduction in ONE custom DVE op (FRAC_RNE_ANT, registered below):
      frac = u - ((u + 1.5*2^23) - 1.5*2^23)  [f32 RNE rounds to nearest
      integer] -> fp16 SBUF. This removes the negative-identity -k matmul
      accumulate from the PE entirely (no extra matmuls, no LDWEIGHTS churn,
      and the PSUM bank is freed right after the DVE pass).
    - ACT sin: s = sin(2*pi*frac) from fp16 SBUF -> fp16 SBUF (natural
      [token, hidden] layout)
    - one 4 MB fp16 DMA out of s per quarter
- Host: y = x_f32 + 2 * s (residual add in f32 on host: removes a device DVE
  pass, halves output DMA vs f32 y, and keeps the residual path exact).
"""

import sys

sys.path.insert(0, "/opt/trn_rl_repo")

import numpy as np

import concourse.bacc as bacc
import concourse.bass as bass
import concourse.dve_ops as dve_ops
import concourse.tile as tile
from concourse import mybir
from concourse.bass import ts
from concourse.bass_utils import run_bass_kernel_spmd
from concourse.dve_ops import DveOp
from concourse.dve_spec import Spec, Src0, C0, C1
from concourse.dve_table_gen import dve_ver_for
from concourse.dve_uop import DveOpSpec

P = 128
HIDDEN = 4096
RANK = 16
N_CORES = 8
TOTAL_ROWS = 4 * 4096
ROWS = TOTAL_ROWS // N_CORES  # 2048 tokens per core
T = 512  # steady-state tokens per quarter (pipeline unit)
TEDGE = 256  # first/last quarter size: halves pipeline fill + drain
KC = HIDDEN // P  # 32 hidden chunks
UBLK = 1024  # hidden block width of one u tile (2 PSUM banks)
NUB = HIDDEN // UBLK  # 4 hidden blocks
MAGIC = 12582912.0  # 1.5 * 2^23: f32 add/sub rounds to nearest integer
SCALE_2PI = 6.283185  # slightly < 2*pi so the Sin LUT arg stays in [-pi, pi]

F32 = mybir.dt.float32
F32R = mybir.dt.float32r
BF16 = mybir.dt.bfloat16
FP16 = mybir.dt.float16


def _frac_ref(in0, in1, s0, s1, imm2):
    a = (in0.astype(np.float32) + np.float32(s0)).astype(np.float32)
    k = (a - np.float32(s1)).astype(np.float32)
    return (in0.astype(np.float32) - k).astype(np.float32)


def _register_frac_op():
    """Register the FRAC_RNE_ANT custom DVE op (one-instruction magic-number
    range reduction: out = in0 - ((in0 + s0) - s1), s0 = s1 = 1.5*2^23)."""
    for op in dve_ops.OPS:
        if op.name == "FRAC_RNE_ANT":
            return op
    spec = Spec(body=Src0 - ((Src0 + C0) - C1), reference=_frac_ref)
    op = DveOp("FRAC_RNE_ANT", spec, subdim=False, uops_sha={})
    dve_ops.OPS.append(op)
    dve_ops.CUSTOM_DVE_SPECS[op.name] = spec
    dve_ops._SUB_OPCODE_FOR_NAME[op.name] = (
        max(dve_ops._SUB_OPCODE_FOR_NAME.values()) + 1
    )
    for trn in ("TRN2",):
        ver = dve_ver_for(trn)
        from concourse.dve_spec import lower

        s = DveOpSpec(
            name=op.name,
            opcode=dve_ops.get_dve_sub_opcode(op.name),
            uops=lower(spec, ver=ver),
            rd1_en=False,
        )
        op.uops_sha[ver] = s.sha(ver)
    return op


FRAC_OP = _register_frac_op()


def _quarter_sizes(rows):
    if rows > 2 * TEDGE and (rows - 2 * TEDGE) % T == 0:
        return [TEDGE] + [T] * ((rows - 2 * TEDGE) // T) + [TEDGE]
    return [T] * (rows // T)


def build_nc(rows: int = ROWS):
    """Per-core Bass program for a [rows, 4096] token shard."""
    sizes = _quarter_sizes(rows)

    nc = bacc.Bacc(
        "TRN2",
        target_bir_lowering=False,
        debug=False,
        enable_asserts=False,
        num_devices=N_CORES,
    )
    # x prearranged on host: [128, KC*rows] fp16; per-quarter blocks of
    # [128, KC*tok] (partition-contiguous), element (p, off_q + k*tok + t) =
    # x[tok0_q + t, k*128 + p] of this core's natural [rows, 4096] shard.
    x_d = nc.dram_tensor("x", [P, KC * rows], FP16, kind="ExternalInput").ap()
    # A prearranged: [128, KC, 16] fp16 (exact int8 values).
    a_d = nc.dram_tensor("A", [P, KC, RANK], FP16, kind="ExternalInput").ap()
    bp_d = nc.dram_tensor("Bp", [RANK, HIDDEN], F32, kind="ExternalInput").ap()
    # s output in NATURAL layout [rows, 4096] fp16.
    s_d = nc.dram_tensor("out", [rows, HIDDEN], FP16, kind="ExternalOutput").ap()

    with tile.TileContext(nc) as tc:
        with (
            tc.tile_pool(name="singles", bufs=1) as singles,
            tc.tile_pool(name="xp", bufs=2) as xpool,
            tc.tile_pool(name="sp", bufs=2) as spool,
            tc.tile_pool(name="fp", bufs=3) as fpool,
            tc.tile_pool(name="h1sb", bufs=2) as h1pool,
            tc.tile_pool(name="h1p", bufs=2, space="PSUM") as h1_psum,
            tc.tile_pool(name="up", bufs=3, space="PSUM") as u_psum,
        ):
            a_sb = singles.tile([P, KC, RANK], FP16)
            nc.sync.dma_start(out=a_sb[:], in_=a_d[:, :, :])
            bp_sb = singles.tile([RANK, HIDDEN], F32R)
            nc.sync.dma_start(out=bp_sb[:], in_=bp_d[:, :].bitcast(F32R))

            def tail_jobs(state):
                """Generator of tail-job closures for a finished quarter."""
                h1_sb, s_sb, _row0, nch = state

                def job(c, ub):
                    u_ps = u_psum.tile([P, UBLK], F32)
                    for jj in range(2):
                        nc.tensor.matmul(
                            u_ps[:, ts(jj, 512)],
                            h1_sb[:, ts(c, P)],
                            bp_sb[:, ub * UBLK + jj * 512 : ub * UBLK + (jj + 1) * 512],
                            start=True,
                            stop=True,
                        )
                    frac = fpool.tile([P, UBLK], FP16)
                    nc.vector._custom_dve(
                        FRAC_OP, out=frac[:], in0=u_ps[:], s0=MAGIC, s1=MAGIC
                    )
                    nc.scalar.activation(
                        out=s_sb[:, c, ts(ub, UBLK)],
                        in_=frac[:],
                        func=mybir.ActivationFunctionType.Sin,
                        scale=SCALE_2PI,
                    )

                for c in range(nch):
                    for ub in range(NUB):
                        yield lambda c=c, ub=ub: job(c, ub)

            def flush_c(prev, c):
                row0, s_sb = prev[2], prev[1]
                r0 = row0 + c * P
                nc.scalar.dma_start(
                    out=s_d[r0 : r0 + P, :].rearrange("(c p) h -> p c h", p=P),
                    in_=s_sb[:, c : c + 1, :],
                )

            prev = None  # (h1_sb, s_sb, row0, nch) of the previous quarter
            prev_jobs = None
            row0 = 0
            for tok in sizes:
                nch = tok // P
                x_sb = xpool.tile([P, KC * tok], FP16)
                off = KC * row0
                npc = 4
                piece = KC * tok // npc
                for pi in range(npc):
                    nc.sync.dma_start(
                        out=x_sb[:, pi * piece : (pi + 1) * piece],
                        in_=x_d[:, off + pi * piece : off + (pi + 1) * piece],
                    )
                s_sb = spool.tile([P, nch, HIDDEN], FP16)
                h1_ps = h1_psum.tile([RANK, tok], F32)
                prev_njobs = prev[3] * NUB if prev is not None else 0
                emitted = 0
                flushed_c = 0
                for k in range(KC):
                    nc.tensor.matmul(
                        h1_ps[:],
                        a_sb[:, k, :],
                        x_sb[:, k * tok : (k + 1) * tok],
                        start=(k == 0),
                        stop=(k == KC - 1),
                    )
                    if prev_jobs is not None:
                        target = (k + 1) * prev_njobs // KC
                        while emitted < target:
                            next(prev_jobs)()
                            emitted += 1
                        while (flushed_c + 1) * NUB <= emitted:
                            flush_c(prev, flushed_c)
                            flushed_c += 1
                h1_sb = h1pool.tile([RANK, tok], F32R)
                nc.vector.tensor_copy(h1_sb[:], h1_ps[:])
                if prev_jobs is not None:
                    while flushed_c < prev[3]:
                        flush_c(prev, flushed_c)
                        flushed_c += 1
                prev = (h1_sb, s_sb, row0, nch)
                prev_jobs = tail_jobs(prev)
                row0 += tok

            # drain: the last quarter's tail has no successor to hide in
            for j, job in enumerate(prev_jobs):
                job()
                if (j + 1) % NUB == 0:
                    flush_c(prev, j // NUB)

    nc.compile()
    return nc


_NC_CACHE: dict[int, object] = {}


def _get_nc(rows: int = ROWS):
    nc = _NC_CACHE.get(rows)
    if nc is None:
        nc = build_nc(rows)
        _NC_CACHE[rows] = nc
    return nc


def _prep_weights(A_int8, B_int8, scale_A, scale_B):
    # A as exact integer values in fp16, prearranged [128, KC, 16]
    a_f = np.ascontiguousarray(
        A_int8.astype(np.float16).reshape(KC, P, RANK).transpose(1, 0, 2)
    )
    bp = np.ascontiguousarray(
        scale_A.astype(np.float32)[:, None]
        * B_int8.astype(np.float32)
        * scale_B.astype(np.float32)[None, :]
        * np.float32(1.0 / (np.pi * np.pi))
    )
    return a_f, bp


def _prearrange_x(x16_shard):
    """[rows, 4096] fp16 -> [128, KC*rows] flat per-quarter blocks."""
    rows = x16_shard.shape[0]
    blocks = []
    r0 = 0
    for tok in _quarter_sizes(rows):
        blk = x16_shard[r0 : r0 + tok].reshape(tok, KC, P).transpose(2, 1, 0)
        blocks.append(np.ascontiguousarray(blk).reshape(P, KC * tok))
        r0 += tok
    return np.ascontiguousarray(np.concatenate(blocks, axis=1))


def kernel(x, A_int8, B_int8, scale_A, scale_B):
    x = np.asarray(x)
    orig_shape = x.shape
    xf = x.reshape(TOTAL_ROWS, HIDDEN)
    x16 = xf.astype(np.float16)
    a_f, bp = _prep_weights(
        np.asarray(A_int8), np.asarray(B_int8), np.asarray(scale_A), np.asarray(scale_B)
    )

    nc = _get_nc(ROWS)
    in_maps = [
        {
            "x": _prearrange_x(x16[i * ROWS : (i + 1) * ROWS]),
            "A": a_f,
            "Bp": bp,
        }
        for i in range(N_CORES)
    ]
    res = run_bass_kernel_spmd(nc, in_maps, core_ids=list(range(N_CORES)))
    y = np.empty((TOTAL_ROWS, HIDDEN), dtype=np.float32)
    for i, r in enumerate(res.results):
        y[i * ROWS : (i + 1) * ROWS] = xf[i * ROWS : (i + 1) * ROWS] + 2.0 * r[
            "out"
        ].astype(np.float32)
    return y.reshape(orig_shape)


# revision 24
# speedup vs baseline: 1.0852x; 1.0032x over previous
"""PiLoraLayer TRN2 kernel: y = x + (alpha/r) * sin((2/pi) * (x @ A) @ B).

x: [4, 4096, 4096] f32; A = A_int8 * scale_A (per-col), B = B_int8 * scale_B
(per-col); rank 16 bottleneck.

Strategy (data-parallel over 8 NeuronCores, fp16 in / fp16-sine out):
- Host: cast x to fp16 and PRE-TRANSPOSE each core's [2048, 4096] token shard
  to hidden-major layout [quarter, partition, k-chunk, token] so the hidden
  dim lands on SBUF partitions for mm1 (no PE transposes, no PSUM->SBUF
  copies of x).
- Host folds scales into Bp = scale_A[:,None] * B_q * scale_B[None,:] / pi^2
  (f32); A stays as exact int values in fp16. Then u = (x@A)@Bp equals
  arg/(2*pi) and y = x + 2*sin(2*pi*u).
- Device per 512-token quarter:
    - one 4 MB fp16 DMA in (partition-contiguous, prearranged on host)
    - mm1 (fp16): h1_ps[16, 512] += A_k^T @ xT_k over 32 hidden chunks;
      DVE copies h1 to SBUF f32r.
    - mm2 (f32r, natural out): per (token-chunk c, hidden-block ub):
      u[128, 1024] = h1_c^T @ Bp_block in two N=512 matmuls.
    - range reduction in ONE custom DVE op (FRAC_RNE_ANT, registered below):
      frac = u - ((u + 1.5*2^23) - 1.5*2^23)  [f32 RNE rounds to nearest
      integer] -> fp16 SBUF. This removes the negative-identity -k matmul
      accumulate from the PE entirely (no extra matmuls, no LDWEIGHTS churn,
      and the PSUM bank is freed right after the DVE pass).
    - ACT sin: s = sin(2*pi*frac) from fp16 SBUF -> fp16 SBUF (natural
      [token, hidden] layout)
    - one 4 MB fp16 DMA out of s per quarter
- Host: y = x_f32 + 2 * s (residual add in f32 on host: removes a device DVE
  pass, halves output DMA vs f32 y, and keeps the residual path exact).
"""

import sys

sys.path.insert(0, "/opt/trn_rl_repo")

import numpy as np

import concourse.bacc as bacc
import concourse.bass as bass
import concourse.dve_ops as dve_ops
import concourse.tile as tile
from concourse import mybir
from concourse.bass import ts
from concourse.bass_utils import run_bass_kernel_spmd
from concourse.dve_ops import DveOp
from concourse.dve_spec import Spec, Src0, C0, C1
from concourse.dve_table_gen import dve_ver_for
from concourse.dve_uop import DveOpSpec

P = 128
HIDDEN = 4096
RANK = 16
N_CORES = 8
TOTAL_ROWS = 4 * 4096
ROWS = TOTAL_ROWS // N_CORES  # 2048 tokens per core
T = 512  # steady-state tokens per quarter (pipeline unit)
TEDGE = 256  # first/last quarter size: halves pipeline fill + drain
KC = HIDDEN // P  # 32 hidden chunks
UBLK = 1024  # hidden block width of one u tile (2 PSUM banks)
NUB = HIDDEN // UBLK  # 4 hidden blocks
MAGIC = 12582912.0  # 1.5 * 2^23: f32 add/sub rounds to nearest integer
SCALE_2PI = 6.283185  # slightly < 2*pi so the Sin LUT arg stays in [-pi, pi]

F32 = mybir.dt.float32
F32R = mybir.dt.float32r
BF16 = mybir.dt.bfloat16
FP16 = mybir.dt.float16


def _frac_ref(in0, in1, s0, s1, imm2):
    a = (in0.astype(np.float32) + np.float32(s0)).astype(np.float32)
    k = (a - np.float32(s1)).astype(np.float32)
    return (in0.astype(np.float32) - k).astype(np.float32)


def _register_frac_op():
    """Register the FRAC_RNE_ANT custom DVE op (one-instruction magic-number
    range reduction: out = in0 - ((in0 + s0) - s1), s0 = s1 = 1.5*2^23)."""
    for op in dve_ops.OPS:
        if op.name == "FRAC_RNE_ANT":
            return op
    spec = Spec(body=Src0 - ((Src0 + C0) - C1), reference=_frac_ref)
    op = DveOp("FRAC_RNE_ANT", spec, subdim=False, uops_sha={})
    dve_ops.OPS.append(op)
    dve_ops.CUSTOM_DVE_SPECS[op.name] = spec
    dve_ops._SUB_OPCODE_FOR_NAME[op.name] = (
        max(dve_ops._SUB_OPCODE_FOR_NAME.values()) + 1
    )
    for trn in ("TRN2",):
        ver = dve_ver_for(trn)
        from concourse.dve_spec import lower

        s = DveOpSpec(
            name=op.name,
            opcode=dve_ops.get_dve_sub_opcode(op.name),
            uops=lower(spec, ver=ver),
            rd1_en=False,
        )
        op.uops_sha[ver] = s.sha(ver)
    return op


FRAC_OP = _register_frac_op()


def _quarter_sizes(rows):
    if rows > 2 * TEDGE and (rows - 2 * TEDGE) % T == 0:
        return [TEDGE] + [T] * ((rows - 2 * TEDGE) // T) + [TEDGE]
    return [T] * (rows // T)


def build_nc(rows: int = ROWS):
    """Per-core Bass program for a [rows, 4096] token shard."""
    sizes = _quarter_sizes(rows)

    nc = bacc.Bacc(
        "TRN2",
        target_bir_lowering=False,
        debug=False,
        enable_asserts=False,
        num_devices=N_CORES,
    )
    # x prearranged on host: [128, KC*rows] fp16; per-quarter blocks of
    # [128, KC*tok] (partition-contiguous), element (p, off_q + k*tok + t) =
    # x[tok0_q + t, k*128 + p] of this core's natural [rows, 4096] shard.
    x_d = nc.dram_tensor("x", [P, KC * rows], FP16, kind="ExternalInput").ap()
    # A prearranged: [128, KC, 16] fp16 (exact int8 values).
    a_d = nc.dram_tensor("A", [P, KC, RANK], FP16, kind="ExternalInput").ap()
    bp_d = nc.dram_tensor("Bp", [RANK, HIDDEN], F32, kind="ExternalInput").ap()
    # s output in NATURAL layout [rows, 4096] fp16.
    s_d = nc.dram_tensor("out", [rows, HIDDEN], FP16, kind="ExternalOutput").ap()

    with tile.TileContext(nc) as tc:
        with (
            tc.tile_pool(name="singles", bufs=1) as singles,
            tc.tile_pool(name="xp", bufs=2) as xpool,
            tc.tile_pool(name="sp", bufs=2) as spool,
            tc.tile_pool(name="fp", bufs=5) as fpool,
            tc.tile_pool(name="h1sb", bufs=2) as h1pool,
            tc.tile_pool(name="h1p", bufs=2, space="PSUM") as h1_psum,
            tc.tile_pool(name="up", bufs=3, space="PSUM") as u_psum,
        ):
            a_sb = singles.tile([P, KC, RANK], FP16)
            nc.sync.dma_start(out=a_sb[:], in_=a_d[:, :, :])
            bp_sb = singles.tile([RANK, HIDDEN], F32R)
            nc.sync.dma_start(out=bp_sb[:], in_=bp_d[:, :].bitcast(F32R))

            def tail_jobs(state):
                """Generator of tail-job closures for a finished quarter."""
                h1_sb, s_sb, _row0, nch = state

                def job(c, ub):
                    u_ps = u_psum.tile([P, UBLK], F32)
                    for jj in range(2):
                        nc.tensor.matmul(
                            u_ps[:, ts(jj, 512)],
                            h1_sb[:, ts(c, P)],
                            bp_sb[:, ub * UBLK + jj * 512 : ub * UBLK + (jj + 1) * 512],
                            start=True,
                            stop=True,
                        )
                    frac = fpool.tile([P, UBLK], FP16)
                    nc.vector._custom_dve(
                        FRAC_OP, out=frac[:], in0=u_ps[:], s0=MAGIC, s1=MAGIC
                    )
                    nc.scalar.activation(
                        out=s_sb[:, c, ts(ub, UBLK)],
                        in_=frac[:],
                        func=mybir.ActivationFunctionType.Sin,
                        scale=SCALE_2PI,
                    )

                for c in range(nch):
                    for ub in range(NUB):
                        yield lambda c=c, ub=ub: job(c, ub)

            def flush_c(prev, c):
                row0, s_sb = prev[2], prev[1]
                r0 = row0 + c * P
                nc.scalar.dma_start(
                    out=s_d[r0 : r0 + P, :].rearrange("(c p) h -> p c h", p=P),
                    in_=s_sb[:, c : c + 1, :],
                )

            prev = None  # (h1_sb, s_sb, row0, nch) of the previous quarter
            prev_jobs = None
            row0 = 0
            for tok in sizes:
                nch = tok // P
                x_sb = xpool.tile([P, KC * tok], FP16)
                off = KC * row0
                npc = 4
                piece = KC * tok // npc
                for pi in range(npc):
                    nc.sync.dma_start(
                        out=x_sb[:, pi * piece : (pi + 1) * piece],
                        in_=x_d[:, off + pi * piece : off + (pi + 1) * piece],
                    )
                s_sb = spool.tile([P, nch, HIDDEN], FP16)
                h1_ps = h1_psum.tile([RANK, tok], F32)
                prev_njobs = prev[3] * NUB if prev is not None else 0
                emitted = 0
                flushed_c = 0
                for k in range(KC):
                    nc.tensor.matmul(
                        h1_ps[:],
                        a_sb[:, k, :],
                        x_sb[:, k * tok : (k + 1) * tok],
                        start=(k == 0),
                        stop=(k == KC - 1),
                    )
                    if prev_jobs is not None:
                        target = (k + 1) * prev_njobs // KC
                        while emitted < target:
                            next(prev_jobs)()
                            emitted += 1
                        while (flushed_c + 1) * NUB <= emitted:
                            flush_c(prev, flushed_c)
                            flushed_c += 1
                h1_sb = h1pool.tile([RANK, tok], F32R)
                nc.vector.tensor_copy(h1_sb[:], h1_ps[:])
                if prev_jobs is not None:
                    while flushed_c < prev[3]:
                        flush_c(prev, flushed_c)
                        flushed_c += 1
                prev = (h1_sb, s_sb, row0, nch)
                prev_jobs = tail_jobs(prev)
                row0 += tok

            # drain: the last quarter's tail has no successor to hide in
            for j, job in enumerate(prev_jobs):
                job()
                if (j + 1) % NUB == 0:
                    flush_c(prev, j // NUB)

    nc.compile()
    return nc


_NC_CACHE: dict[int, object] = {}


def _get_nc(rows: int = ROWS):
    nc = _NC_CACHE.get(rows)
    if nc is None:
        nc = build_nc(rows)
        _NC_CACHE[rows] = nc
    return nc


def _prep_weights(A_int8, B_int8, scale_A, scale_B):
    # A as exact integer values in fp16, prearranged [128, KC, 16]
    a_f = np.ascontiguousarray(
        A_int8.astype(np.float16).reshape(KC, P, RANK).transpose(1, 0, 2)
    )
    bp = np.ascontiguousarray(
        scale_A.astype(np.float32)[:, None]
        * B_int8.astype(np.float32)
        * scale_B.astype(np.float32)[None, :]
        * np.float32(1.0 / (np.pi * np.pi))
    )
    return a_f, bp


def _prearrange_x(x16_shard):
    """[rows, 4096] fp16 -> [128, KC*rows] flat per-quarter blocks."""
    rows = x16_shard.shape[0]
    blocks = []
    r0 = 0
    for tok in _quarter_sizes(rows):
        blk = x16_shard[r0 : r0 + tok].reshape(tok, KC, P).transpose(2, 1, 0)
        blocks.append(np.ascontiguousarray(blk).reshape(P, KC * tok))
        r0 += tok
    return np.ascontiguousarray(np.concatenate(blocks, axis=1))


def kernel(x, A_int8, B_int8, scale_A, scale_B):
    x = np.asarray(x)
    orig_shape = x.shape
    xf = x.reshape(TOTAL_ROWS, HIDDEN)
    x16 = xf.astype(np.float16)
    a_f, bp = _prep_weights(
        np.asarray(A_int8), np.asarray(B_int8), np.asarray(scale_A), np.asarray(scale_B)
    )

    nc = _get_nc(ROWS)
    in_maps = [
        {
            "x": _prearrange_x(x16[i * ROWS : (i + 1) * ROWS]),
            "A": a_f,
            "Bp": bp,
        }
        for i in range(N_CORES)
    ]
    res = run_bass_kernel_spmd(nc, in_maps, core_ids=list(range(N_CORES)))
    y = np.empty((TOTAL_ROWS, HIDDEN), dtype=np.float32)
    for i, r in enumerate(res.results):
        y[i * ROWS : (i + 1) * ROWS] = xf[i * ROWS : (i + 1) * ROWS] + 2.0 * r[
            "out"
        ].astype(np.float32)
    return y.reshape(orig_shape)
